# revision 26
# baseline (speedup 1.0000x reference)
"""RPE (relative-position-bias) attention kernel for Trainium2, 8-core SPMD.

Full op (per reference):
  qkv = x @ w_qkv.T -> split q,k,v heads (H=12, hd=64), q *= hd**-0.5
  attn = q @ k.T ; attn[:, :, 1:, 1:] += rpb_table[rel_idx]  (per head)
  attn = softmax(attn, -1) ; out = (attn @ v) @ w_proj.T + b_proj

Sharding: data-parallel over batch. B=64 -> 8 batches per core. Weights
and bias-derived planes replicated to all cores. No collectives.

Per-core program (all matmuls bf16 operands, fp32 PSUM accumulation):
  - Inputs arrive bf16 and pre-transposed from the host: xT [768,1576],
    wqkvT [768,2304] (q columns pre-scaled by hd**-0.5), wpT [768,768].
  - qT,kT [768,1576] = w_chunk.T @ xT (transposed layout). v in natural
    layout [tokens, head, 65] with a ones column (softmax denominators
    fall out of the AV matmul for free).
  - The relative-position bias enters as exp(bias): probs = exp(s) *
    expb, where expb planes are host-precomputed bf16 in the transposed
    orientation [k_tok, q_tok] per head PAIR (two heads side by side,
    394 columns). exp runs on the scalar engine straight out of PSUM;
    the expb multiply runs on gpsimd in SBUF, keeping DVE light and
    releasing PSUM banks early.
  - Heads are processed in pairs: score tiles [128,394] hold two heads.
  - Softmax normalization: denominators live in row 64 of the AV PSUM
    tile; 1/denom via DVE reciprocal, then a tiny f32 matmul
    (E.T @ rec2, E = 2x128 block-ones) broadcasts the two heads' recs
    across 128 partitions -- no DRAM bounce, no dynamic DMA.
  - out = attn_outT.T @ wpT + b_proj  (fp32 output).
"""
import sys

sys.path.insert(0, '/opt/trn_rl_repo')

from contextlib import ExitStack

import numpy as np

import concourse.bass as bass
import concourse.bacc as bacc
import concourse.tile as tile
from concourse import mybir

# ---- problem dims (hardcoded per contract) ----
NCORES = 8
B_FULL = 64
B = B_FULL // NCORES     # 8 batches per core
N = 197                  # tokens (196 patches + CLS)
NP = 196
C = 768
H = 12
HD = 64
R = B * N                # 1576 rows per core
NPAIR = H // 2           # 6 head pairs
W2 = 2 * N               # 394 columns for a head pair

F32 = mybir.dt.float32
BF16 = mybir.dt.bfloat16
AF = mybir.ActivationFunctionType

import os
STAGE = int(os.environ.get("KERNEL_STAGE", "6"))
NO_INTERLEAVE = int(os.environ.get("KERNEL_NO_INTERLEAVE", "0"))
# CoreSim rejects reads of uninitialized PSUM; the exp deliberately reads a
# dead corner of the score tile (rows 69:128 of the chunk-1 columns, never
# consumed downstream). Sim runs memset it; hardware runs skip the cost.
SIM_SAFE = int(os.environ.get("KERNEL_SIM_SAFE", "0"))


def _scalar_recip(nc, out, in_):
    """Scalar-engine reciprocal via direct InstActivation emission. The
    public activation() API refuses AF.Reciprocal over worst-case accuracy;
    measured on hardware it is ~1e-5 max rel err for positive O(100)
    softmax denominators, which is far inside this kernel's budget, and it
    is ~4x cheaper than the DVE reciprocal for row-shaped operands."""
    ins = [nc.scalar.lower_ap(in_)]
    for val in (0.0, 1.0, 0.0):
        ins.append(mybir.ImmediateValue(dtype=mybir.dt.float32, value=val))
    return nc.scalar.add_instruction(mybir.InstActivation(
        name=nc.get_next_instruction_name(),
        func=AF.Reciprocal, ins=ins,
        outs=[nc.scalar.lower_ap(out)]))


def build_program():
    nc = bacc.Bacc("TRN2", target_bir_lowering=False, debug=False)

    x_d = nc.declare_dram_parameter("xT", [C, R], BF16, isOutput=False)
    wqkv_d = nc.declare_dram_parameter("w_qkvT", [C, 3 * C], BF16, isOutput=False)
    wp_d = nc.declare_dram_parameter("w_projT", [C, C], BF16, isOutput=False)
    bp_d = nc.declare_dram_parameter("b_proj", [C], F32, isOutput=False)
    # exp(bias) planes per head, transposed chunk-paired orientation:
    # expb [head, k_part 0:128, q 0:197 (k chunk 0) ++ q 0:197 (k chunk 1)]
    # (chunk 1 rows beyond k=196 are 1.0 and multiply unused garbage)
    expb_d = nc.declare_dram_parameter("expb", [H, 128, W2], BF16,
                                       isOutput=False)
    out_d = nc.declare_dram_parameter("out", [R, C], BF16, isOutput=True)

    with tile.TileContext(nc) as tc:
        with ExitStack() as ctx:
            _emit(ctx, tc, nc, x_d, wqkv_d, wp_d, bp_d, expb_d, out_d)
    nc.compile()
    return nc


def _emit(ctx, tc, nc, x_d, wqkv_d, wp_d, bp_d, expb_d, out_d):
    singles = ctx.enter_context(tc.tile_pool(name="singles", bufs=1))
    ps_pool = ctx.enter_context(tc.tile_pool(name="ps", bufs=4, space="PSUM"))
    av_ps = ctx.enter_context(tc.tile_pool(name="av_ps", bufs=4, space="PSUM"))
    probs_pool = ctx.enter_context(tc.tile_pool(name="probs", bufs=8))
    small_pool = ctx.enter_context(tc.tile_pool(name="small", bufs=4))
    out_pool = ctx.enter_context(tc.tile_pool(name="outp", bufs=4))

    KC = C // 128  # 6 contraction chunks

    # ---------------- load operands (already bf16 + transposed) ----------
    xT = []     # 6 x [128, R] bf16
    wqkvT = []  # 6 x [128, 2304] bf16
    wpT = []    # 6 x [128, 768] bf16
    for kc in range(KC):
        t = singles.tile([128, R], BF16, tag=f"xT{kc}", name=f"xT{kc}")
        nc.gpsimd.dma_start(out=t[:], in_=x_d[128 * kc:128 * (kc + 1), :])
        xT.append(t)
        w = singles.tile([128, 3 * C], BF16, tag=f"wqkvT{kc}", name=f"wqkvT{kc}")
        nc.sync.dma_start(out=w[:, 0:2 * C],
                          in_=wqkv_d[128 * kc:128 * (kc + 1), 0:2 * C])
        wqkvT.append(w)
    for kc in range(KC):
        nc.gpsimd.dma_start(out=wqkvT[kc][:, 2 * C:3 * C],
                            in_=wqkv_d[128 * kc:128 * (kc + 1), 2 * C:3 * C])
    for kc in range(KC):
        t = singles.tile([128, C], BF16, tag=f"wpT{kc}", name=f"wpT{kc}")
        nc.sync.dma_start(out=t[:], in_=wp_d[128 * kc:128 * (kc + 1), :])
        wpT.append(t)

    bproj_bc = singles.tile([128, C], F32, tag="bproj")
    nc.gpsimd.dma_start(out=bproj_bc[:],
                        in_=bass.AP(tensor=bp_d, offset=0, ap=[[0, 128], [1, C]]))

    expb = []  # [128, 394] bf16 per head (chunk-paired columns)
    for h in range(H):
        t0 = singles.tile([128, W2], BF16, tag=f"expb_{h}", name=f"expb_{h}")
        nc.sync.dma_start(out=t0[:], in_=expb_d[h, :, :])
        expb.append(t0)

    def _dummy_out():
        zt = out_pool.tile([128, C], BF16, tag="out", name="zdump")
        nc.vector.memset(zt[:], 0.0)
        nc.sync.dma_start(out=out_d[0:128, :], in_=zt[:])

    if STAGE <= 1:
        _dummy_out()
        return

    # ---------------- QKV ----------------
    NCHUNK = 4
    NW = R // NCHUNK  # 394 columns per psum tile

    qk_sb = [None] * 12  # 0..5 = qT feature chunks (head pair p), 6..11 = kT

    def emit_qk(ft):
        dst = singles.tile([128, R], BF16, tag=f"qk{ft}", name=f"qk{ft}")
        qk_sb[ft] = dst
        tiles = [ps_pool.tile([128, NW], F32, tag="ps", name=f"qkps{ncol}")
                 for ncol in range(NCHUNK)]
        for kc in range(KC):
            for ncol in range(NCHUNK):
                nc.tensor.matmul(
                    out=tiles[ncol][:],
                    lhsT=wqkvT[kc][:, 128 * ft:128 * (ft + 1)],
                    rhs=xT[kc][:, NW * ncol:NW * (ncol + 1)],
                    start=(kc == 0), stop=(kc == KC - 1))
        for ncol in range(NCHUNK):
            nc.vector.tensor_copy(
                out=dst[:, NW * ncol:NW * (ncol + 1)], in_=tiles[ncol][:])

    # v_aug[b][c]: [128, 12, 65] bf16 (col 64 = ones)
    v_aug = [[None, None] for _ in range(B)]

    def emit_v():
        for b in range(B):
            for cchunk, (r0, nr) in enumerate(((N * b, 128), (N * b + 128, N - 128))):
                dst = singles.tile([128, H, HD + 1], BF16, tag=f"v{b}_{cchunk}",
                                   name=f"v{b}_{cchunk}")
                v_aug[b][cchunk] = dst
                nc.vector.memset(dst[:, :, HD:HD + 1], 1.0)
                for nh in range(2):
                    ps = ps_pool.tile([128, 384], F32, tag="ps")
                    for kc in range(KC):
                        nc.tensor.matmul(
                            out=ps[:nr, :],
                            lhsT=xT[kc][:, r0:r0 + nr],
                            rhs=wqkvT[kc][:, 2 * C + 384 * nh:2 * C + 384 * (nh + 1)],
                            start=(kc == 0), stop=(kc == KC - 1))
                    nc.vector.tensor_copy(
                        out=dst[:nr, 6 * nh:6 * (nh + 1), 0:HD],
                        in_=ps[:nr, :].rearrange("p (h d) -> p h d", h=6))

    # deferred normalize closures (one per attention wave)
    pending_norm = []

    def flush_norm():
        while pending_norm:
            pending_norm.pop(0)()

    # attn output, transposed: 6 tiles [128, R] bf16 (pair p = heads 2p,2p+1)
    attn_outT = []
    for p in range(NPAIR):
        attn_outT.append(singles.tile([128, R], BF16, tag=f"aoT{p}",
                                      name=f"aoT{p}"))

    def emit_attention_pair(p, waves=(0, 1)):
        N1 = N - 128  # 69
        qTp = qk_sb[p]
        kTp = qk_sb[6 + p]
        dst = attn_outT[p]
        # waves of 4 batches: all 4 reciprocals run back-to-back on the
        # scalar engine, so the Exp<->Reciprocal activation-table reloads
        # (1.3us each) amortize over the wave instead of every iteration
        for wave in waves:
            avs = []
            c0s = []

            def emit_scores(j):
                b = 4 * wave + j
                c0 = N * b
                c0s.append(c0)
                qh = [qTp[0:64, c0:c0 + N], qTp[64:128, c0:c0 + N]]
                kh = [kTp[0:64, c0:c0 + N], kTp[64:128, c0:c0 + N]]
                phs = []
                for hh in range(2):
                    sth = ps_pool.tile([128, W2], F32, tag="ps",
                                       name=f"sth{hh}")
                    if SIM_SAFE:
                        nc.vector.memset(sth[64:128, N:W2], 0.0)
                    nc.tensor.matmul(out=sth[:, 0:N],
                                     lhsT=kh[hh][:, 0:128], rhs=qh[hh],
                                     start=True, stop=True)
                    nc.tensor.matmul(out=sth[0:N1, N:W2],
                                     lhsT=kh[hh][:, 128:N], rhs=qh[hh],
                                     start=True, stop=True)
                    ph = probs_pool.tile([128, W2], BF16, tag="probs")
                    nc.scalar.activation(out=ph[:], in_=sth[:], func=AF.Exp)
                    if STAGE >= 4:
                        nc.vector.tensor_mul(out=ph[:], in0=ph[:],
                                             in1=expb[2 * p + hh][:])
                    phs.append(ph)
                return phs

            def emit_av(j, phs):
                b = 4 * wave + j
                av = av_ps.tile([HD + 1, W2], F32, tag="av")
                avs.append(av)
                for hh in range(2):
                    h = 2 * p + hh
                    nc.tensor.matmul(out=av[:, N * hh:N * hh + N],
                                     lhsT=v_aug[b][0][:, h, :],
                                     rhs=phs[hh][:, 0:N],
                                     start=True, stop=False)
                    nc.tensor.matmul(out=av[:, N * hh:N * hh + N],
                                     lhsT=v_aug[b][1][0:N1, h, :],
                                     rhs=phs[hh][0:N1, N:W2],
                                     start=False, stop=True)

            pending = [emit_scores(0), emit_scores(1)]
            # previous wave's normalize runs here: its reciprocals land on
            # the scalar queue AFTER this wave's first exps, so the exps hit
            # a warm Exp table and the first AV matmul is not gated behind a
            # Recip->Exp activation-table reload
            flush_norm()
            for j in range(4):
                emit_av(j, pending[j])
                if j + 2 < 4:
                    pending.append(emit_scores(j + 2))

            if STAGE <= 4:
                for j in range(4):
                    nc.scalar.activation(out=dst[0:64, c0s[j]:c0s[j] + N],
                                         in_=avs[j][0:HD, 0:N], func=AF.Copy)
                    nc.scalar.activation(out=dst[64:128, c0s[j]:c0s[j] + N],
                                         in_=avs[j][0:HD, N:W2], func=AF.Copy)
                continue

            def norm(avs=avs, c0s=list(c0s), dst=dst):
                recs = []
                for j in range(4):
                    rec2 = small_pool.tile([1, W2], F32, tag="rec2")
                    _scalar_recip(nc, rec2[0:1, :], avs[j][HD:HD + 1, :])
                    recs.append(rec2)
                for j in range(4):
                    rec_sb = small_pool.tile([128, W2], F32, tag="rec_sb")
                    nc.gpsimd.partition_broadcast(rec_sb[:], recs[j][0:1, :])
                    nc.vector.tensor_mul(out=dst[0:64, c0s[j]:c0s[j] + N],
                                         in0=avs[j][0:HD, 0:N],
                                         in1=rec_sb[0:64, 0:N])
                    nc.vector.tensor_mul(out=dst[64:128, c0s[j]:c0s[j] + N],
                                         in0=avs[j][0:HD, N:W2],
                                         in1=rec_sb[64:128, N:W2])
            pending_norm.append(norm)

    # emission order: first qk pair + v, then attention per pair interleaved
    # with the remaining qk pairs, so V/S/GpSimd overlap the T-bound qkv.
    if NO_INTERLEAVE:
        for ft in range(12):
            emit_qk(ft)
        emit_v()
        if STAGE <= 2:
            _dummy_out()
            return
        for p in range(NPAIR - 1):
            emit_attention_pair(p)
        last_pair_split = STAGE >= 3
    else:
        emit_qk(0)
        emit_qk(6)
        emit_v()
        if STAGE <= 2:
            _dummy_out()
            return
        for p in range(NPAIR - 1):
            if p > 0:
                emit_qk(p)
                emit_qk(6 + p)
            if STAGE >= 3:
                emit_attention_pair(p)
        emit_qk(NPAIR - 1)
        emit_qk(6 + NPAIR - 1)
        last_pair_split = STAGE >= 3

    if STAGE <= 3:
        if last_pair_split:
            emit_attention_pair(NPAIR - 1)
        flush_norm()
        _dummy_out()
        return

    # ---------------- proj ----------------
    NRC = (R + 127) // 128  # 13 row chunks

    def emit_proj(rcs):
        for rc in rcs:
            r0 = 128 * rc
            nr = min(128, R - r0)
            for nh in range(2):
                ps = ps_pool.tile([128, 384], F32, tag="ps")
                for kc in range(KC):
                    nc.tensor.matmul(
                        out=ps[:nr, :],
                        lhsT=attn_outT[kc][:, r0:r0 + nr],
                        rhs=wpT[kc][:, 384 * nh:384 * (nh + 1)],
                        start=(kc == 0), stop=(kc == KC - 1))
                ot = out_pool.tile([128, 384], BF16, tag="out")
                nc.vector.tensor_add(out=ot[:nr, :], in0=ps[:nr, :],
                                     in1=bproj_bc[:nr, 384 * nh:384 * (nh + 1)])
                nc.sync.dma_start(
                    out=out_d[r0:r0 + nr, 384 * nh:384 * (nh + 1)],
                    in_=ot[:nr, :])
    # last pair: wave 0 (batches 0-3), then the proj row-chunks those
    # batches complete, then wave 1, then the rest -- shrinks the tail
    emit_attention_pair(NPAIR - 1, waves=(0,))
    flush_norm()
    emit_proj(range(0, 6))
    emit_attention_pair(NPAIR - 1, waves=(1,))
    flush_norm()
    emit_proj(range(6, NRC))


_NC_CACHE = {}


def _get_nc():
    if "nc" not in _NC_CACHE:
        _NC_CACHE["nc"] = build_program()
    return _NC_CACHE["nc"]


def prep_aux(rpb_table, rel_idx):
    """Host-side prep: gather the bias from the two small aux inputs, lay it
    out per head PAIR in the kernel's transposed plane orientation
    [k_tok, q_tok*2] with zeroed CLS row/col, and exponentiate (bf16)."""
    import ml_dtypes
    bT = rpb_table[rel_idx.reshape(-1)].reshape(NP, NP, H)  # [q_idx, k_idx, h]
    bT = np.ascontiguousarray(bT.transpose(1, 0, 2))        # [k_idx, q_idx, h]
    bias0 = np.zeros((128, N, H), dtype=np.float32)
    bias0[1:128, 1:NP + 1, :] = bT[0:127]
    bias1 = np.zeros((128, N, H), dtype=np.float32)
    bias1[0:NP - 127, 1:NP + 1, :] = bT[127:NP]
    expb = np.zeros((H, 128, W2), dtype=np.float32)
    for h in range(H):
        expb[h, :, 0:N] = np.exp(bias0[:, :, h])
        expb[h, :, N:W2] = np.exp(bias1[:, :, h])
    return expb.astype(ml_dtypes.bfloat16)


def prep_weights(w_qkv, w_proj):
    """Host-side prep: transpose, fold the q scale into w_qkv, cast bf16."""
    import ml_dtypes
    wqkvT = np.array(w_qkv, dtype=np.float32).T.copy()
    wqkvT[:, 0:C] *= HD ** -0.5
    wpT = np.ascontiguousarray(np.asarray(w_proj, dtype=np.float32).T)
    return (wqkvT.astype(ml_dtypes.bfloat16), wpT.astype(ml_dtypes.bfloat16))


def make_in_maps(x, w_qkv, w_proj, b_proj, rpb_table, rel_idx):
    """Build the 8 per-core input maps (host prep: shard, transpose, bf16)."""
    import ml_dtypes
    x = np.asarray(x, dtype=np.float32)
    expb = prep_aux(
        np.asarray(rpb_table, dtype=np.float32), np.asarray(rel_idx).astype(np.int64))
    wqkvT, wpT = prep_weights(w_qkv, w_proj)
    bp = np.ascontiguousarray(np.asarray(b_proj, dtype=np.float32))
    xbf = x.astype(ml_dtypes.bfloat16)
    in_maps = []
    for c in range(NCORES):
        xT = np.ascontiguousarray(xbf[c * B:(c + 1) * B].reshape(R, C).T)
        in_maps.append({
            "xT": xT,
            "w_qkvT": wqkvT,
            "w_projT": wpT,
            "b_proj": bp,
            "expb": expb,
        })
    return in_maps


def kernel(x, w_qkv, w_proj, b_proj, rpb_table, rel_idx):
    from concourse.bass_utils import run_bass_kernel_spmd

    nc = _get_nc()
    in_maps = make_in_maps(x, w_qkv, w_proj, b_proj, rpb_table, rel_idx)
    res = run_bass_kernel_spmd(nc, in_maps, list(range(NCORES)))
    out = np.concatenate(
        [np.asarray(r["out"], dtype=np.float32).reshape(B, N, C)
         for r in res.results], axis=0)
    return out
